# revision 16
# baseline (speedup 1.0000x reference)
"""Trainium2 Bass kernel for a dense transformer block (pre-LN, causal MHA + FFN).

Sharding: pure data-parallel over batch — 8 sequences -> 8 NeuronCores, no
collectives. Each core runs the full block on its [2048, 400] slice.

Design is calibrated to measured per-instruction PE costs: weight loads
hide under accumulation chains but cost ~135 ns on single-group matmuls
with a fresh lhsT, and DoubleRow's 256-wide weight load only pays off for
2-long chains that replace 4 bf16 chunks. Hence:
  - q/k/v/proj: fp8-e4m3 DoubleRow 2-chains (contraction 400->2x(101,2)).
  - scores: fp8 single-group matmuls, i-major (kT block stationary across
    j) so consecutive groups reuse the loaded weights; exp(logit/16-ln16)
    on ACT -> probsT fp8 (1/16 scale cancels in softmax, keeps exp <= 48
    under the e4m3 max of 240). Causal masking on Pool in SBUF (memzero +
    0/1 triangular multiply) — no PSUM mask adds.
  - attn@V: row-form fp8 single accumulation chains over s-blocks; ones
    column of v1 gives the softmax denominator; rows scaled by 4/denom ->
    bf16 -> PE transpose -> aoT fp8.
  - fc1/fc2: plain bf16 accumulation chains (weight loads hide), h2/ffT
    bf16. LN gains and every bias are folded host-side; biases enter as
    rank-1 terms through ones rows kept at contraction index 100 (h/h2/ao)
    or 64 (ffT chunk 12).
LN: stats on DVE, normalize on Pool, PE transposes, merged psum->SBUF
copies. Residual stream stays f32 in SBUF.

All weight reshaping/casting is host-side numpy, shipped as ExternalInputs.
"""

import math

import numpy as np
import ml_dtypes

import concourse.bass as bass
import concourse.mybir as mybir
import concourse.tile as tile
from concourse import bacc
from concourse.bass_utils import run_bass_kernel_spmd

BF16NP = ml_dtypes.bfloat16
E4NP = ml_dtypes.float8_e4m3
BF16 = mybir.dt.bfloat16
E4 = mybir.dt.float8e4
F32 = mybir.dt.float32
AF = mybir.ActivationFunctionType
ALU = mybir.AluOpType
DR = mybir.MatmulPerfMode.DoubleRow

P = 128          # partitions
B = 8            # batch -> cores
T = 2048         # sequence length
C = 400          # embed dim
H = 4            # heads
D = 100          # head dim
K1 = D + 1       # contraction rows incl. ones row (bias rank-1 term)
DFF = 1600       # ffn hidden
NT = T // P      # 16 row tiles
WT = 512         # wide tile for qk matmuls
TJ = 512         # t-tile width for transposed attention scores
NTJ = T // TJ    # 4
FT = 512         # ffn column-slice width
NFT = T // FT    # 4
NFC = (DFF + P - 1) // P  # 13 f-chunks (12x128 + 64)
LN16 = math.log(16.0)

LAST_RESULT = None  # BassKernelResults of the most recent run (for test.py)


def build_block(loop_n=None):
    nc = bacc.Bacc("TRN2", target_bir_lowering=False, debug=False)

    x_d = nc.dram_tensor("x", [T, C], F32, kind="ExternalInput")
    wq_d = nc.dram_tensor("wqp", [P, H, 2, 2, P], E4, kind="ExternalInput")
    wk_d = nc.dram_tensor("wkp", [P, H, 2, 2, P], E4, kind="ExternalInput")
    wv_d = nc.dram_tensor("wvp", [P, 2, 2, C], E4, kind="ExternalInput")
    wo_d = nc.dram_tensor("wop", [P, 2, 2, C], E4, kind="ExternalInput")
    w1_d = nc.dram_tensor("w1p", [P, H, DFF], BF16, kind="ExternalInput")
    w2_d = nc.dram_tensor("w2p", [P, NFC, C], BF16, kind="ExternalInput")
    tri_d = nc.dram_tensor("trip", [P, P], E4, kind="ExternalInput")
    id_d = nc.dram_tensor("identp", [P, P], BF16, kind="ExternalInput")
    out_d = nc.dram_tensor("out", [T, C], F32, kind="ExternalOutput")

    with tile.TileContext(nc) as tc:
        with (
            tc.tile_pool(name="consts", bufs=1) as consts,
            tc.tile_pool(name="persist", bufs=1) as persist,
            tc.tile_pool(name="qk", bufs=2) as qk_pool,
            tc.tile_pool(name="pr", bufs=2) as pr_pool,
            tc.tile_pool(name="work", bufs=3) as work,
            tc.tile_pool(name="arows", bufs=8) as arow_pool,
            tc.tile_pool(name="small", bufs=4) as small,
            tc.tile_pool(name="ps_sc", bufs=2, space="PSUM") as ps_sc,
            tc.tile_pool(name="ps_tr", bufs=2, space="PSUM") as ps_tr,
            tc.tile_pool(name="ps_acc", bufs=2, space="PSUM") as ps_acc,
        ):
            # ---- constants into SBUF (one-time) ----
            def cload(tag, dram, shape, dtype):
                t_ = consts.tile(shape, dtype, tag=tag, name=tag)
                nc.sync.dma_start(t_[:], dram[:])
                return t_

            wq_sb = cload("wq", wq_d, [P, H, 2, 2, P], E4)
            wk_sb = cload("wk", wk_d, [P, H, 2, 2, P], E4)
            wv_sb = cload("wv", wv_d, [P, 2, 2, C], E4)
            wo_sb = cload("wo", wo_d, [P, 2, 2, C], E4)
            w1_sb = cload("w1", w1_d, [P, H, DFF], BF16)
            w2_sb = cload("w2", w2_d, [P, NFC, C], BF16)
            tri_sb = cload("tri", tri_d, [P, P], E4)
            id_sb = cload("ident", id_d, [P, P], BF16)
            eps_sb = consts.tile([P, 1], F32, tag="eps", name="eps")
            nc.vector.memset(eps_sb, 1e-5)
            nln16_sb = consts.tile([P, 1], F32, tag="nln16", name="nln16")
            nc.vector.memset(nln16_sb, -LN16)

            # ---- persistent tiles + one-time init of constant rows ----
            x_sb = persist.tile([P, NT, C], F32, tag="x", name="x_sb")
            hT = persist.tile([P, H, T], E4, tag="hT", name="hT")
            h2T = persist.tile([P, H, T], BF16, tag="h2T", name="h2T")
            v1 = persist.tile([P, NT, H, D + 2], E4, tag="v1", name="v1")
            ao = persist.tile([P, H, T], E4, tag="ao", name="ao")
            ffa = persist.tile([P, NFC, FT], BF16, tag="ffa", name="ffa")
            ffb = persist.tile([P, NFC, FT], BF16, tag="ffb", name="ffb")

            # partition-base must be 0/32/64/96: set [96:128]; rows 96..99
            # are rewritten by the per-iteration copies before any matmul
            # reads them, rows 101.. are never read (K1=101 contractions).
            nc.vector.memset(hT[96:, :, :], 1.0)
            nc.vector.memset(h2T[96:, :, :], 1.0)
            nc.vector.memset(ao[96:, :, :], 1.0)
            nc.vector.memset(v1[:, :, :, D], 1.0)
            nc.vector.memset(v1[:, :, :, D + 1], 0.0)
            nc.vector.memset(ffa[64:96, NFC - 1, :], 1.0)
            nc.vector.memset(ffb[64:96, NFC - 1, :], 1.0)

            xr = x_d.rearrange("(n p) c -> p n c", p=P)
            outr = out_d.rearrange("(n p) c -> p n c", p=P)

            def layernorm(dst, cb=None):
                """LN over x_sb row tiles; g/b are folded into the weights
                host-side, so this is the plain (x-mu)*rstd. Result bf16 ->
                PE-transposed into dst [100, cc, T] (fp8 hT or bf16 h2T)."""
                mv = small.tile([P, NT, 2], F32, tag="mv", name="mv")
                for ti in range(NT):
                    stats = small.tile([P, 6], F32, tag="stats", name="stats")
                    nc.vector.bn_stats(out=stats, in_=x_sb[:, ti, :])
                    nc.vector.bn_aggr(out=mv[:, ti, :], in_=stats)
                rstd = small.tile([P, NT], F32, tag="rstd", name="rstd")
                nc.scalar.activation(out=rstd, in_=mv[:, :, 1], func=AF.Sqrt,
                                     bias=eps_sb, scale=1.0)
                nc.vector.reciprocal(out=rstd, in_=rstd)
                for ti in range(NT):
                    hbf = work.tile([P, C], BF16, tag="hbf", name="hbf")
                    nc.vector.tensor_scalar(
                        out=hbf, in0=x_sb[:, ti, :],
                        scalar1=mv[:, ti, 0:1], scalar2=rstd[:, ti:ti + 1],
                        op0=ALU.subtract, op1=ALU.mult)
                    ptr = ps_tr.tile([P, H, 2 * P], BF16, tag="tr", name="ptr")
                    for cc in range(H):
                        nc.tensor.transpose(
                            ptr[:D, cc, :P], hbf[:, cc * D:(cc + 1) * D], id_sb)
                    nc.vector.tensor_copy(
                        out=dst[:D, :, ti * P:(ti + 1) * P],
                        in_=ptr[:D, :, :P])
                    if cb is not None:
                        cb(ti)

            def body():
                # ---- x into SBUF per row tile ----
                for ti in range(NT):
                    nc.sync.dma_start(x_sb[:, ti, :], xr[:, ti, :])

                # ---- LN1 -> hT (fp8), V matmuls interleaved per tile ----
                def emit_v(ti):
                    psv = ps_sc.tile([P, 2 * WT], F32, tag="sc", name="psv")
                    for p_ in range(2):
                        nc.tensor.matmul(
                            psv[:, :C],
                            lhsT=hT[:K1, 2 * p_:2 * p_ + 2, ti * P:(ti + 1) * P],
                            rhs=wv_sb[:K1, p_, :, :],
                            start=(p_ == 0), stop=(p_ == 1), perf_mode=DR)
                    nc.vector.tensor_scalar_mul(
                        out=v1[:, ti, :, :D],
                        in0=psv[:, :C].rearrange("p (h d) -> p h d", h=H),
                        scalar1=1.0 / 16.0)

                def emit_qk(qT, kT, h, tt):
                    sl = slice(tt * WT, (tt + 1) * WT)
                    psqk = ps_sc.tile([P, 2 * WT], F32, tag="sc",
                                      name="psqk")
                    for p_ in range(2):
                        nc.tensor.matmul(
                            psqk[:D, :WT],
                            lhsT=wq_sb[:K1, h, p_, :, :D],
                            rhs=hT[:K1, 2 * p_:2 * p_ + 2, sl],
                            start=(p_ == 0), stop=(p_ == 1), perf_mode=DR)
                    for p_ in range(2):
                        nc.tensor.matmul(
                            psqk[:D, WT:],
                            lhsT=wk_sb[:K1, h, p_, :, :D],
                            rhs=hT[:K1, 2 * p_:2 * p_ + 2, sl],
                            start=(p_ == 0), stop=(p_ == 1), perf_mode=DR)
                    # qT holds 16*(0.1 q), kT holds k; exp divides by 16
                    nc.vector.tensor_scalar_mul(
                        out=qT[:D, sl], in0=psqk[:D, :WT], scalar1=0.1)
                    nc.scalar.mul(out=kT[:D, sl], in_=psqk[:D, WT:],
                                  mul=1.0 / 16.0)


                qk0 = (qk_pool.tile([P, T], E4, tag="qT", name="qT"),
                       qk_pool.tile([P, T], E4, tag="kT", name="kT"))

                def ln1_cb(ti):
                    emit_v(ti)
                    if ti % 4 == 3:
                        emit_qk(qk0[0], qk0[1], 0, ti // 4)

                layernorm(hT, cb=ln1_cb)

                # ---- per-head attention ----
                # scores are i-major (kT block stationary across j); attn@V
                # is row-form fp8 chains, software-pipelined one i-quad
                # behind the scores/exp producer.
                def attnv_chains(pjT_, h_, q_):
                    arows = []
                    for tc_ in range(4):
                        ti = 4 * q_ + tc_
                        pso = ps_acc.tile([P, WT], F32, tag="acc", name="pso")
                        for si in range(ti + 1):
                            nc.tensor.matmul(
                                pso[:, :D + 2],
                                lhsT=pjT_[:, si, q_, tc_ * P:(tc_ + 1) * P],
                                rhs=v1[:, si, h_, :],
                                start=(si == 0), stop=(si == ti))
                        rec = small.tile([P, 1], F32, tag="rec", name="rec")
                        nc.vector.reciprocal(out=rec, in_=pso[:, D:D + 1])
                        arow = arow_pool.tile([P, D], BF16, tag="arow",
                                              name="arow")
                        nc.vector.tensor_scalar_mul(
                            out=arow, in0=pso[:, :D], scalar1=rec)
                        arows.append(arow)
                    return arows

                def attnv_transposes(arows, h_, q_):
                    # runs one quad behind the chains so the PE queue never
                    # waits on the DVE rec/arow round-trip
                    pta = ps_tr.tile([P, H, 2 * P], BF16, tag="tr", name="pta")
                    for tc_ in range(4):
                        nc.tensor.transpose(pta[:D, tc_, :P], arows[tc_],
                                            id_sb)
                    nc.vector.tensor_scalar_mul(
                        out=ao[:D, h_, q_ * TJ:(q_ + 1) * TJ],
                        in0=pta[:D, :, :P], scalar1=4.0)

                pend_tr = None
                pend_av = None

                for h in range(H):
                    if h == 0:
                        qT, kT = qk0
                    else:
                        qT = qk_pool.tile([P, T], E4, tag="qT", name="qT")
                        kT = qk_pool.tile([P, T], E4, tag="kT", name="kT")
                        for tt in range(T // WT):
                            emit_qk(qT, kT, h, tt)

                    pjT = pr_pool.tile([P, NT, NTJ, TJ], E4, tag="probsT",
                                       name="pjT")
                    for q in range(4):
                        for i in range(4 * q, 4 * q + 4):
                            jlist = list(range(i // 4, NTJ))
                            for x0 in range(0, len(jlist), 2):
                                jpair = jlist[x0:x0 + 2]
                                w = len(jpair)
                                pss = ps_sc.tile([P, 2 * WT], F32, tag="sc",
                                                 name="pss")
                                for u, j in enumerate(jpair):
                                    nc.tensor.matmul(
                                        pss[:, u * TJ:(u + 1) * TJ],
                                        lhsT=kT[:D, i * P:(i + 1) * P],
                                        rhs=qT[:D, j * TJ:(j + 1) * TJ],
                                        start=True, stop=True)
                                nc.scalar.activation(
                                    out=pjT[:, i, jpair[0]:jpair[0] + w, :],
                                    in_=pss[:, :w * TJ].rearrange(
                                        "p (a b) -> p a b", a=w),
                                    func=AF.Exp, bias=nln16_sb,
                                    scale=1.0 / 16.0)
                            # causal masking in SBUF (Pool)
                            jd, r = i // 4, i % 4
                            if r:
                                nc.vector.memzero(pjT[:, i, jd, :r * P])
                            dv = pjT[:, i, jd, r * P:(r + 1) * P]
                            nc.vector.tensor_mul(out=dv, in0=dv, in1=tri_sb)
                        if pend_av is not None:
                            ph, hh, qq = pend_av
                            ar = attnv_chains(ph, hh, qq)
                            if pend_tr is not None:
                                attnv_transposes(*pend_tr)
                            pend_tr = (ar, hh, qq)
                        pend_av = (pjT, h, q)

                if pend_av is not None:
                    ph, hh, qq = pend_av
                    ar = attnv_chains(ph, hh, qq)
                    if pend_tr is not None:
                        attnv_transposes(*pend_tr)
                    pend_tr = (ar, hh, qq)
                if pend_tr is not None:
                    attnv_transposes(*pend_tr)

                # ---- output projection (fp8 DR head pairs) + residual ----
                for ti in range(NT):
                    pg = ps_acc.tile([P, WT], F32, tag="acc", name="pgp")
                    for p_ in range(2):
                        nc.tensor.matmul(
                            pg[:, :C],
                            lhsT=ao[:K1, 2 * p_:2 * p_ + 2,
                                    ti * P:(ti + 1) * P],
                            rhs=wo_sb[:K1, p_, :, :],
                            start=(p_ == 0), stop=(p_ == 1), perf_mode=DR)
                    nc.vector.scalar_tensor_tensor(
                        out=x_sb[:, ti, :], in0=pg[:, :C], scalar=1.0 / 64.0,
                        in1=x_sb[:, ti, :], op0=ALU.mult, op1=ALU.add)

                # ---- LN2 -> h2T (bf16) with fc1/fc2 interleaved ----
                def emit_fc2(ffT_, ft):
                    for tl in range(FT // P):
                        ti = ft * (FT // P) + tl
                        pg = ps_acc.tile([P, WT], F32, tag="acc", name="pgf")
                        for ch in range(NFC):
                            fsz = P if ch < NFC - 1 else (DFF - (NFC - 1) * P
                                                          + 1)
                            nc.tensor.matmul(
                                pg[:, :C],
                                lhsT=ffT_[:fsz, ch, tl * P:(tl + 1) * P],
                                rhs=w2_sb[:fsz, ch, :],
                                start=(ch == 0), stop=(ch == NFC - 1))
                        orow = work.tile([P, C], F32, tag="orow", name="orow")
                        nc.vector.scalar_tensor_tensor(
                            out=orow, in0=pg[:, :C], scalar=1.0,
                            in1=x_sb[:, ti, :], op0=ALU.mult, op1=ALU.add)
                        nc.sync.dma_start(outr[:, ti, :], orow)

                def emit_fc1(ft):
                    ffT_ = ffa if ft % 2 == 0 else ffb
                    slf = slice(ft * FT, (ft + 1) * FT)
                    for fp_ in range(7):
                        psf = ps_sc.tile([P, 2 * FT], F32, tag="sc",
                                         name="psf")
                        nch = 2 if fp_ < 6 else 1
                        for u in range(nch):
                            fc = 2 * fp_ + u
                            fsz = min(P, DFF - fc * P)
                            for cc in range(H):
                                nc.tensor.matmul(
                                    psf[:fsz, u * FT:(u + 1) * FT],
                                    lhsT=w1_sb[:K1, cc,
                                               fc * P:fc * P + fsz],
                                    rhs=h2T[:K1, cc, slf],
                                    start=(cc == 0), stop=(cc == H - 1))
                        if fp_ < 6:
                            if fp_ % 2 == 0:
                                nc.scalar.activation(
                                    out=ffT_[:, 2 * fp_:2 * fp_ + 2, :],
                                    in_=psf[:, :].rearrange(
                                        "p (a b) -> p a b", a=2),
                                    func=AF.Relu, bias=0.0, scale=1.0)
                            else:
                                nc.vector.tensor_scalar_max(
                                    out=ffT_[:, 2 * fp_:2 * fp_ + 2, :],
                                    in0=psf[:, :].rearrange(
                                        "p (a b) -> p a b", a=2),
                                    scalar1=0.0)
                        else:
                            fsz = DFF - (NFC - 1) * P
                            nc.scalar.activation(
                                out=ffT_[:fsz, NFC - 1, :],
                                in_=psf[:fsz, :FT], func=AF.Relu, bias=0.0,
                                scale=1.0)
                    return ffT_

                pend_fc2 = [None]

                def ln2_cb(ti):
                    if ti % 4 == 3:
                        ft = ti // 4
                        ffT_ = emit_fc1(ft)
                        if pend_fc2[0] is not None:
                            emit_fc2(*pend_fc2[0])
                        pend_fc2[0] = (ffT_, ft)

                layernorm(h2T, cb=ln2_cb)
                emit_fc2(*pend_fc2[0])

            if loop_n is None:
                body()
            else:
                with tc.For_i(0, loop_n, 1):
                    body()

    nc.finalize()
    return nc


def prep_weights(Wq, Wk, Wv, Wo, bo, W1, b1, W2, b2,
                 ln1_g, ln1_b, ln2_g, ln2_b):
    """Host-side reshape/cast into the layouts the device program expects.

    LN gains fold into the weights (LN runs plain on device); LN betas and
    linear biases become rank-1 terms fed by the ones rows the device keeps
    at contraction index 100 (hT/h2T/ao) or 64 of chunk 12 (ffT)."""
    f = np.float32
    Wq = np.asarray(Wq, f); Wk = np.asarray(Wk, f); Wv = np.asarray(Wv, f)
    Wo = np.asarray(Wo, f); W1 = np.asarray(W1, f); W2 = np.asarray(W2, f)
    bo = np.asarray(bo, f); b1 = np.asarray(b1, f); b2 = np.asarray(b2, f)
    g1 = np.asarray(ln1_g, f); be1 = np.asarray(ln1_b, f)
    g2 = np.asarray(ln2_g, f); be2 = np.asarray(ln2_b, f)

    gWq = g1[None, :, None] * Wq      # [H, C, D]
    gWk = g1[None, :, None] * Wk
    gWv = g1[None, :, None] * Wv
    gW1 = g2[:, None] * W1            # [C, DFF]
    bq = np.einsum("c,hcd->hd", be1, Wq)   # beta1 contributions
    bk = np.einsum("c,hcd->hd", be1, Wk)
    bv = np.einsum("c,hcd->hd", be1, Wv)
    b1f = b1 + be2 @ W1

    def qkpack(gW, bias):
        w = np.zeros((P, H, 2, 2, P), E4NP)
        for h in range(H):
            for p in range(2):
                for s in range(2):
                    cc = 2 * p + s
                    w[:D, h, p, s, :D] = (
                        16.0 * gW[h, cc * D:(cc + 1) * D, :]).astype(E4NP)
            w[D, h, 0, 0, :D] = (16.0 * bias[h]).astype(E4NP)
        return w

    wqp = qkpack(gWq, bq)
    wkp = qkpack(gWk, bk)

    # V: columns ordered (h, d) to match v1's [h, d] split
    wvp = np.zeros((P, 2, 2, C), E4NP)
    Wvf = gWv.transpose(1, 0, 2).reshape(C, C)   # [c, (h d)]
    for p in range(2):
        for s in range(2):
            cc = 2 * p + s
            wvp[:D, p, s, :] = (16.0 * Wvf[cc * D:(cc + 1) * D, :]
                                ).astype(E4NP)
    wvp[D, 0, 0, :] = (16.0 * bv.reshape(C)).astype(E4NP)

    # proj: slices are heads (din within head on partitions); aoT carries 4x
    # scaled rows plus a ones row, psum = 64*(proj + bo)
    wop = np.zeros((P, 2, 2, C), E4NP)
    for p in range(2):
        for s in range(2):
            hh = 2 * p + s
            wop[:D, p, s, :] = (16.0 * Wo[hh * D:(hh + 1) * D, :]
                                ).astype(E4NP)
    wop[D, 0, 0, :] = (64.0 * bo).astype(E4NP)

    # fc1: bf16, bias row at [100, cc=0]
    w1p = np.zeros((P, H, DFF), BF16NP)
    for cc in range(H):
        w1p[:D, cc, :] = gW1[cc * D:(cc + 1) * D, :].astype(BF16NP)
    w1p[D, 0, :] = b1f.astype(BF16NP)

    # fc2: bf16, b2 via the ffT ones row at partition 64 of chunk 12
    w2p = np.zeros((P, NFC, C), BF16NP)
    for ch in range(NFC):
        fsz = min(P, DFF - ch * P)
        w2p[:fsz, ch, :] = W2[ch * P:ch * P + fsz, :].astype(BF16NP)
    w2p[P // 2, NFC - 1, :] = b2.astype(BF16NP)

    sl_ = np.arange(P)[:, None]
    tl_ = np.arange(P)[None, :]
    trip = np.where(tl_ >= sl_, 1.0, 0.0).astype(E4NP)
    ident = np.eye(P, dtype=BF16NP)
    return {
        "wqp": wqp, "wkp": wkp, "wvp": wvp, "wop": wop,
        "w1p": w1p, "w2p": w2p,
        "trip": np.ascontiguousarray(trip), "identp": ident,
    }


_CACHED_NC = None


def kernel(x, ln1_g, ln1_b, ln2_g, ln2_b, Wq, Wk, Wv, Wo, bo, W1, b1, W2, b2,
           trace=False):
    global _CACHED_NC, LAST_RESULT
    x = np.asarray(x, np.float32)
    assert x.shape == (B, T, C), x.shape
    wmap = prep_weights(Wq, Wk, Wv, Wo, bo, W1, b1, W2, b2,
                        ln1_g, ln1_b, ln2_g, ln2_b)
    if _CACHED_NC is None:
        _CACHED_NC = build_block()
    nc = _CACHED_NC
    in_maps = [dict(wmap, x=np.ascontiguousarray(x[c])) for c in range(B)]
    res = run_bass_kernel_spmd(nc, in_maps, core_ids=list(range(B)),
                               trace=trace)
    LAST_RESULT = res
    out = np.stack([res.results[c]["out"] for c in range(B)])
    return out.astype(np.float32)


# revision 17
# speedup vs baseline: 1.2063x; 1.2063x over previous
"""Trainium2 Bass kernel for a dense transformer block (pre-LN, causal MHA + FFN).

Sharding: pure data-parallel over batch — 8 sequences -> 8 NeuronCores, no
collectives. Each core runs the full block on its [2048, 400] slice.

Design is calibrated to measured per-instruction PE costs: weight loads
hide under accumulation chains but cost ~135 ns on single-group matmuls
with a fresh lhsT, and DoubleRow's 256-wide weight load only pays off for
2-long chains that replace 4 bf16 chunks. Hence:
  - q/k/v/proj: fp8-e4m3 DoubleRow 2-chains (contraction 400->2x(101,2)).
  - scores: fp8 single-group matmuls, i-major (kT block stationary across
    j) so consecutive groups reuse the loaded weights; exp(logit/16-ln16)
    on ACT -> probsT fp8 (1/16 scale cancels in softmax, keeps exp <= 48
    under the e4m3 max of 240). Causal masking on Pool in SBUF (memzero +
    0/1 triangular multiply) — no PSUM mask adds.
  - attn@V: row-form fp8 single accumulation chains over s-blocks; ones
    column of v1 gives the softmax denominator; rows scaled by 4/denom ->
    bf16 -> PE transpose -> aoT fp8.
  - fc1/fc2: plain bf16 accumulation chains (weight loads hide), h2/ffT
    bf16. LN gains and every bias are folded host-side; biases enter as
    rank-1 terms through ones rows kept at contraction index 100 (h/h2/ao)
    or 64 (ffT chunk 12).
LN: stats on DVE, normalize on Pool, PE transposes, merged psum->SBUF
copies. Residual stream stays f32 in SBUF.

All weight reshaping/casting is host-side numpy, shipped as ExternalInputs.
"""

import math

import numpy as np
import ml_dtypes

import concourse.bass as bass
import concourse.mybir as mybir
import concourse.tile as tile
from concourse import bacc
from concourse.bass_utils import run_bass_kernel_spmd

BF16NP = ml_dtypes.bfloat16
E4NP = ml_dtypes.float8_e4m3
BF16 = mybir.dt.bfloat16
E4 = mybir.dt.float8e4
F32 = mybir.dt.float32
AF = mybir.ActivationFunctionType
ALU = mybir.AluOpType
DR = mybir.MatmulPerfMode.DoubleRow

P = 128          # partitions
B = 8            # batch -> cores
T = 2048         # sequence length
C = 400          # embed dim
H = 4            # heads
D = 100          # head dim
K1 = D + 1       # contraction rows incl. ones row (bias rank-1 term)
DFF = 1600       # ffn hidden
NT = T // P      # 16 row tiles
WT = 512         # wide tile for qk matmuls
TJ = 512         # t-tile width for transposed attention scores
NTJ = T // TJ    # 4
FT = 512         # ffn column-slice width
NFT = T // FT    # 4
NFC = (DFF + P - 1) // P  # 13 f-chunks (12x128 + 64)
LN16 = math.log(16.0)

LAST_RESULT = None  # BassKernelResults of the most recent run (for test.py)


def build_block(loop_n=None):
    nc = bacc.Bacc("TRN2", target_bir_lowering=False, debug=False)

    x_d = nc.dram_tensor("x", [T, C], F32, kind="ExternalInput")
    wq_d = nc.dram_tensor("wqp", [P, H, 2, 2, P], E4, kind="ExternalInput")
    wk_d = nc.dram_tensor("wkp", [P, H, 2, 2, P], E4, kind="ExternalInput")
    wv_d = nc.dram_tensor("wvp", [P, 2, 2, C], E4, kind="ExternalInput")
    wo_d = nc.dram_tensor("wop", [P, 2, 2, C], E4, kind="ExternalInput")
    w1_d = nc.dram_tensor("w1p", [P, H, DFF], BF16, kind="ExternalInput")
    w2_d = nc.dram_tensor("w2p", [P, NFC, C], BF16, kind="ExternalInput")
    tri_d = nc.dram_tensor("trip", [P, P], E4, kind="ExternalInput")
    id_d = nc.dram_tensor("identp", [P, P], BF16, kind="ExternalInput")
    out_d = nc.dram_tensor("out", [T, C], F32, kind="ExternalOutput")

    with tile.TileContext(nc) as tc:
        with (
            tc.tile_pool(name="consts", bufs=1) as consts,
            tc.tile_pool(name="persist", bufs=1) as persist,
            tc.tile_pool(name="qk", bufs=2) as qk_pool,
            tc.tile_pool(name="pr", bufs=2) as pr_pool,
            tc.tile_pool(name="work", bufs=3) as work,
            tc.tile_pool(name="arows", bufs=8) as arow_pool,
            tc.tile_pool(name="small", bufs=4) as small,
            tc.tile_pool(name="ps_sc", bufs=2, space="PSUM") as ps_sc,
            tc.tile_pool(name="ps_tr", bufs=2, space="PSUM") as ps_tr,
            tc.tile_pool(name="ps_acc", bufs=2, space="PSUM") as ps_acc,
        ):
            # ---- constants into SBUF (one-time) ----
            def cload(tag, dram, shape, dtype):
                t_ = consts.tile(shape, dtype, tag=tag, name=tag)
                nc.sync.dma_start(t_[:], dram[:])
                return t_

            wq_sb = cload("wq", wq_d, [P, H, 2, 2, P], E4)
            wk_sb = cload("wk", wk_d, [P, H, 2, 2, P], E4)
            wv_sb = cload("wv", wv_d, [P, 2, 2, C], E4)
            wo_sb = cload("wo", wo_d, [P, 2, 2, C], E4)
            w1_sb = cload("w1", w1_d, [P, H, DFF], BF16)
            w2_sb = cload("w2", w2_d, [P, NFC, C], BF16)
            tri_sb = cload("tri", tri_d, [P, P], E4)
            id_sb = cload("ident", id_d, [P, P], BF16)
            eps_sb = consts.tile([P, 1], F32, tag="eps", name="eps")
            nc.vector.memset(eps_sb, 1e-5)
            nln16_sb = consts.tile([P, 1], F32, tag="nln16", name="nln16")
            nc.vector.memset(nln16_sb, -LN16)

            # ---- persistent tiles + one-time init of constant rows ----
            x_sb = persist.tile([P, NT, C], F32, tag="x", name="x_sb")
            hT = persist.tile([P, H, T], E4, tag="hT", name="hT")
            h2T = persist.tile([P, H, T], BF16, tag="h2T", name="h2T")
            v1 = persist.tile([P, NT, H, D + 2], E4, tag="v1", name="v1")
            ao = persist.tile([P, H, T], E4, tag="ao", name="ao")
            ffa = persist.tile([P, NFC, FT], BF16, tag="ffa", name="ffa")
            ffb = persist.tile([P, NFC, FT], BF16, tag="ffb", name="ffb")

            # partition-base must be 0/32/64/96: set [96:128]; rows 96..99
            # are rewritten by the per-iteration copies before any matmul
            # reads them, rows 101.. are never read (K1=101 contractions).
            nc.vector.memset(hT[96:, :, :], 1.0)
            nc.vector.memset(h2T[96:, :, :], 1.0)
            nc.vector.memset(ao[96:, :, :], 1.0)
            nc.vector.memset(v1[:, :, :, D], 1.0)
            nc.vector.memset(v1[:, :, :, D + 1], 0.0)
            nc.vector.memset(ffa[64:96, NFC - 1, :], 1.0)
            nc.vector.memset(ffb[64:96, NFC - 1, :], 1.0)

            xr = x_d.rearrange("(n p) c -> p n c", p=P)
            outr = out_d.rearrange("(n p) c -> p n c", p=P)

            def layernorm(dst, cb=None):
                """LN over x_sb row tiles; g/b are folded into the weights
                host-side, so this is the plain (x-mu)*rstd. Result bf16 ->
                PE-transposed into dst [100, cc, T] (fp8 hT or bf16 h2T)."""
                mv = small.tile([P, NT, 2], F32, tag="mv", name="mv")
                for ti in range(NT):
                    stats = small.tile([P, 6], F32, tag="stats", name="stats")
                    nc.vector.bn_stats(out=stats, in_=x_sb[:, ti, :])
                    nc.vector.bn_aggr(out=mv[:, ti, :], in_=stats)
                rstd = small.tile([P, NT], F32, tag="rstd", name="rstd")
                nc.scalar.activation(out=rstd, in_=mv[:, :, 1], func=AF.Sqrt,
                                     bias=eps_sb, scale=1.0)
                nc.vector.reciprocal(out=rstd, in_=rstd)
                for ti in range(NT):
                    hbf = work.tile([P, C], BF16, tag="hbf", name="hbf")
                    nc.vector.tensor_scalar(
                        out=hbf, in0=x_sb[:, ti, :],
                        scalar1=mv[:, ti, 0:1], scalar2=rstd[:, ti:ti + 1],
                        op0=ALU.subtract, op1=ALU.mult)
                    ptr = ps_tr.tile([P, H, 2 * P], BF16, tag="tr", name="ptr")
                    for cc in range(H):
                        nc.tensor.transpose(
                            ptr[:D, cc, :P], hbf[:, cc * D:(cc + 1) * D], id_sb)
                    nc.vector.tensor_copy(
                        out=dst[:D, :, ti * P:(ti + 1) * P],
                        in_=ptr[:D, :, :P])
                    if cb is not None:
                        cb(ti)

            def body():
                # ---- x into SBUF per row tile ----
                for ti in range(NT):
                    nc.sync.dma_start(x_sb[:, ti, :], xr[:, ti, :])

                # ---- LN1 -> hT (fp8), V matmuls interleaved per tile ----
                def emit_v(ti):
                    psv = ps_sc.tile([P, 2 * WT], F32, tag="sc", name="psv")
                    for p_ in range(2):
                        nc.tensor.matmul(
                            psv[:, :C],
                            lhsT=hT[:K1, 2 * p_:2 * p_ + 2, ti * P:(ti + 1) * P],
                            rhs=wv_sb[:K1, p_, :, :],
                            start=(p_ == 0), stop=(p_ == 1), perf_mode=DR)
                    nc.vector.tensor_scalar_mul(
                        out=v1[:, ti, :, :D],
                        in0=psv[:, :C].rearrange("p (h d) -> p h d", h=H),
                        scalar1=1.0 / 16.0)

                layernorm(hT, cb=emit_v)

                # ---- per-head attention ----
                # scores are i-major (kT block stationary across j); attn@V
                # is row-form fp8 chains, software-pipelined one i-quad
                # behind the scores/exp producer.
                def attnv_chains(pjT_, h_, q_):
                    arows = []
                    for tc_ in range(4):
                        ti = 4 * q_ + tc_
                        pso = ps_acc.tile([P, WT], F32, tag="acc", name="pso")
                        for si in range(ti + 1):
                            nc.tensor.matmul(
                                pso[:, :D + 2],
                                lhsT=pjT_[:, si, q_, tc_ * P:(tc_ + 1) * P],
                                rhs=v1[:, si, h_, :],
                                start=(si == 0), stop=(si == ti))
                        rec = small.tile([P, 1], F32, tag="rec", name="rec")
                        nc.vector.reciprocal(out=rec, in_=pso[:, D:D + 1])
                        arow = arow_pool.tile([P, D], BF16, tag="arow",
                                              name="arow")
                        nc.vector.tensor_scalar_mul(
                            out=arow, in0=pso[:, :D], scalar1=rec)
                        arows.append(arow)
                    return arows

                def attnv_transposes(arows, h_, q_):
                    # runs one quad behind the chains so the PE queue never
                    # waits on the DVE rec/arow round-trip
                    pta = ps_tr.tile([P, H, 2 * P], BF16, tag="tr", name="pta")
                    for tc_ in range(4):
                        nc.tensor.transpose(pta[:D, tc_, :P], arows[tc_],
                                            id_sb)
                    nc.vector.tensor_scalar_mul(
                        out=ao[:D, h_, q_ * TJ:(q_ + 1) * TJ],
                        in0=pta[:D, :, :P], scalar1=4.0)

                pend_tr = None
                pend_av = None
                for h in range(H):
                    qT = qk_pool.tile([P, T], E4, tag="qT", name="qT")
                    kT = qk_pool.tile([P, T], E4, tag="kT", name="kT")
                    for tt in range(T // WT):
                        sl = slice(tt * WT, (tt + 1) * WT)
                        psqk = ps_sc.tile([P, 2 * WT], F32, tag="sc",
                                          name="psqk")
                        for p_ in range(2):
                            nc.tensor.matmul(
                                psqk[:D, :WT],
                                lhsT=wq_sb[:K1, h, p_, :, :D],
                                rhs=hT[:K1, 2 * p_:2 * p_ + 2, sl],
                                start=(p_ == 0), stop=(p_ == 1), perf_mode=DR)
                        for p_ in range(2):
                            nc.tensor.matmul(
                                psqk[:D, WT:],
                                lhsT=wk_sb[:K1, h, p_, :, :D],
                                rhs=hT[:K1, 2 * p_:2 * p_ + 2, sl],
                                start=(p_ == 0), stop=(p_ == 1), perf_mode=DR)
                        # qT holds 16*(0.1 q), kT holds k; exp divides by 16
                        nc.vector.tensor_scalar_mul(
                            out=qT[:D, sl], in0=psqk[:D, :WT], scalar1=0.1)
                        nc.scalar.mul(out=kT[:D, sl], in_=psqk[:D, WT:],
                                      mul=1.0 / 16.0)

                    pjT = pr_pool.tile([P, NT, NTJ, TJ], E4, tag="probsT",
                                       name="pjT")
                    for q in range(4):
                        for i in range(4 * q, 4 * q + 4):
                            jlist = list(range(i // 4, NTJ))
                            for x0 in range(0, len(jlist), 2):
                                jpair = jlist[x0:x0 + 2]
                                w = len(jpair)
                                pss = ps_sc.tile([P, 2 * WT], F32, tag="sc",
                                                 name="pss")
                                for u, j in enumerate(jpair):
                                    nc.tensor.matmul(
                                        pss[:, u * TJ:(u + 1) * TJ],
                                        lhsT=kT[:D, i * P:(i + 1) * P],
                                        rhs=qT[:D, j * TJ:(j + 1) * TJ],
                                        start=True, stop=True)
                                nc.scalar.activation(
                                    out=pjT[:, i, jpair[0]:jpair[0] + w, :],
                                    in_=pss[:, :w * TJ].rearrange(
                                        "p (a b) -> p a b", a=w),
                                    func=AF.Exp, bias=nln16_sb,
                                    scale=1.0 / 16.0)
                            # causal masking in SBUF (Pool)
                            jd, r = i // 4, i % 4
                            if r:
                                nc.vector.memzero(pjT[:, i, jd, :r * P])
                            dv = pjT[:, i, jd, r * P:(r + 1) * P]
                            nc.vector.tensor_mul(out=dv, in0=dv, in1=tri_sb)
                        if pend_av is not None:
                            ph, hh, qq = pend_av
                            ar = attnv_chains(ph, hh, qq)
                            if pend_tr is not None:
                                attnv_transposes(*pend_tr)
                            pend_tr = (ar, hh, qq)
                        pend_av = (pjT, h, q)

                if pend_av is not None:
                    ph, hh, qq = pend_av
                    ar = attnv_chains(ph, hh, qq)
                    if pend_tr is not None:
                        attnv_transposes(*pend_tr)
                    pend_tr = (ar, hh, qq)
                if pend_tr is not None:
                    attnv_transposes(*pend_tr)

                # ---- output projection (fp8 DR head pairs) + residual ----
                for ti in range(NT):
                    pg = ps_acc.tile([P, WT], F32, tag="acc", name="pgp")
                    for p_ in range(2):
                        nc.tensor.matmul(
                            pg[:, :C],
                            lhsT=ao[:K1, 2 * p_:2 * p_ + 2,
                                    ti * P:(ti + 1) * P],
                            rhs=wo_sb[:K1, p_, :, :],
                            start=(p_ == 0), stop=(p_ == 1), perf_mode=DR)
                    nc.vector.scalar_tensor_tensor(
                        out=x_sb[:, ti, :], in0=pg[:, :C], scalar=1.0 / 64.0,
                        in1=x_sb[:, ti, :], op0=ALU.mult, op1=ALU.add)

                # ---- LN2 -> h2T (bf16) with fc1/fc2 interleaved ----
                def emit_fc2(ffT_, ft):
                    for tl in range(FT // P):
                        ti = ft * (FT // P) + tl
                        pg = ps_acc.tile([P, WT], F32, tag="acc", name="pgf")
                        for ch in range(NFC):
                            fsz = P if ch < NFC - 1 else (DFF - (NFC - 1) * P
                                                          + 1)
                            nc.tensor.matmul(
                                pg[:, :C],
                                lhsT=ffT_[:fsz, ch, tl * P:(tl + 1) * P],
                                rhs=w2_sb[:fsz, ch, :],
                                start=(ch == 0), stop=(ch == NFC - 1))
                        orow = work.tile([P, C], F32, tag="orow", name="orow")
                        nc.vector.scalar_tensor_tensor(
                            out=orow, in0=pg[:, :C], scalar=1.0,
                            in1=x_sb[:, ti, :], op0=ALU.mult, op1=ALU.add)
                        nc.sync.dma_start(outr[:, ti, :], orow)

                def emit_fc1(ft):
                    ffT_ = ffa if ft % 2 == 0 else ffb
                    slf = slice(ft * FT, (ft + 1) * FT)
                    for fp_ in range(7):
                        psf = ps_sc.tile([P, 2 * FT], F32, tag="sc",
                                         name="psf")
                        nch = 2 if fp_ < 6 else 1
                        for u in range(nch):
                            fc = 2 * fp_ + u
                            fsz = min(P, DFF - fc * P)
                            for cc in range(H):
                                nc.tensor.matmul(
                                    psf[:fsz, u * FT:(u + 1) * FT],
                                    lhsT=w1_sb[:K1, cc,
                                               fc * P:fc * P + fsz],
                                    rhs=h2T[:K1, cc, slf],
                                    start=(cc == 0), stop=(cc == H - 1))
                        if fp_ < 6:
                            if fp_ % 2 == 0:
                                nc.scalar.activation(
                                    out=ffT_[:, 2 * fp_:2 * fp_ + 2, :],
                                    in_=psf[:, :].rearrange(
                                        "p (a b) -> p a b", a=2),
                                    func=AF.Relu, bias=0.0, scale=1.0)
                            else:
                                nc.vector.tensor_scalar_max(
                                    out=ffT_[:, 2 * fp_:2 * fp_ + 2, :],
                                    in0=psf[:, :].rearrange(
                                        "p (a b) -> p a b", a=2),
                                    scalar1=0.0)
                        else:
                            fsz = DFF - (NFC - 1) * P
                            nc.scalar.activation(
                                out=ffT_[:fsz, NFC - 1, :],
                                in_=psf[:fsz, :FT], func=AF.Relu, bias=0.0,
                                scale=1.0)
                    return ffT_

                pend_fc2 = [None]

                def ln2_cb(ti):
                    if ti % 4 == 3:
                        ft = ti // 4
                        ffT_ = emit_fc1(ft)
                        if pend_fc2[0] is not None:
                            emit_fc2(*pend_fc2[0])
                        pend_fc2[0] = (ffT_, ft)

                layernorm(h2T, cb=ln2_cb)
                emit_fc2(*pend_fc2[0])

            if loop_n is None:
                body()
            else:
                with tc.For_i(0, loop_n, 1):
                    body()

    nc.finalize()
    return nc


def prep_weights(Wq, Wk, Wv, Wo, bo, W1, b1, W2, b2,
                 ln1_g, ln1_b, ln2_g, ln2_b):
    """Host-side reshape/cast into the layouts the device program expects.

    LN gains fold into the weights (LN runs plain on device); LN betas and
    linear biases become rank-1 terms fed by the ones rows the device keeps
    at contraction index 100 (hT/h2T/ao) or 64 of chunk 12 (ffT)."""
    f = np.float32
    Wq = np.asarray(Wq, f); Wk = np.asarray(Wk, f); Wv = np.asarray(Wv, f)
    Wo = np.asarray(Wo, f); W1 = np.asarray(W1, f); W2 = np.asarray(W2, f)
    bo = np.asarray(bo, f); b1 = np.asarray(b1, f); b2 = np.asarray(b2, f)
    g1 = np.asarray(ln1_g, f); be1 = np.asarray(ln1_b, f)
    g2 = np.asarray(ln2_g, f); be2 = np.asarray(ln2_b, f)

    gWq = g1[None, :, None] * Wq      # [H, C, D]
    gWk = g1[None, :, None] * Wk
    gWv = g1[None, :, None] * Wv
    gW1 = g2[:, None] * W1            # [C, DFF]
    bq = np.einsum("c,hcd->hd", be1, Wq)   # beta1 contributions
    bk = np.einsum("c,hcd->hd", be1, Wk)
    bv = np.einsum("c,hcd->hd", be1, Wv)
    b1f = b1 + be2 @ W1

    def qkpack(gW, bias):
        w = np.zeros((P, H, 2, 2, P), E4NP)
        for h in range(H):
            for p in range(2):
                for s in range(2):
                    cc = 2 * p + s
                    w[:D, h, p, s, :D] = (
                        16.0 * gW[h, cc * D:(cc + 1) * D, :]).astype(E4NP)
            w[D, h, 0, 0, :D] = (16.0 * bias[h]).astype(E4NP)
        return w

    wqp = qkpack(gWq, bq)
    wkp = qkpack(gWk, bk)

    # V: columns ordered (h, d) to match v1's [h, d] split
    wvp = np.zeros((P, 2, 2, C), E4NP)
    Wvf = gWv.transpose(1, 0, 2).reshape(C, C)   # [c, (h d)]
    for p in range(2):
        for s in range(2):
            cc = 2 * p + s
            wvp[:D, p, s, :] = (16.0 * Wvf[cc * D:(cc + 1) * D, :]
                                ).astype(E4NP)
    wvp[D, 0, 0, :] = (16.0 * bv.reshape(C)).astype(E4NP)

    # proj: slices are heads (din within head on partitions); aoT carries 4x
    # scaled rows plus a ones row, psum = 64*(proj + bo)
    wop = np.zeros((P, 2, 2, C), E4NP)
    for p in range(2):
        for s in range(2):
            hh = 2 * p + s
            wop[:D, p, s, :] = (16.0 * Wo[hh * D:(hh + 1) * D, :]
                                ).astype(E4NP)
    wop[D, 0, 0, :] = (64.0 * bo).astype(E4NP)

    # fc1: bf16, bias row at [100, cc=0]
    w1p = np.zeros((P, H, DFF), BF16NP)
    for cc in range(H):
        w1p[:D, cc, :] = gW1[cc * D:(cc + 1) * D, :].astype(BF16NP)
    w1p[D, 0, :] = b1f.astype(BF16NP)

    # fc2: bf16, b2 via the ffT ones row at partition 64 of chunk 12
    w2p = np.zeros((P, NFC, C), BF16NP)
    for ch in range(NFC):
        fsz = min(P, DFF - ch * P)
        w2p[:fsz, ch, :] = W2[ch * P:ch * P + fsz, :].astype(BF16NP)
    w2p[P // 2, NFC - 1, :] = b2.astype(BF16NP)

    sl_ = np.arange(P)[:, None]
    tl_ = np.arange(P)[None, :]
    trip = np.where(tl_ >= sl_, 1.0, 0.0).astype(E4NP)
    ident = np.eye(P, dtype=BF16NP)
    return {
        "wqp": wqp, "wkp": wkp, "wvp": wvp, "wop": wop,
        "w1p": w1p, "w2p": w2p,
        "trip": np.ascontiguousarray(trip), "identp": ident,
    }


_CACHED_NC = None


def kernel(x, ln1_g, ln1_b, ln2_g, ln2_b, Wq, Wk, Wv, Wo, bo, W1, b1, W2, b2,
           trace=False):
    global _CACHED_NC, LAST_RESULT
    x = np.asarray(x, np.float32)
    assert x.shape == (B, T, C), x.shape
    wmap = prep_weights(Wq, Wk, Wv, Wo, bo, W1, b1, W2, b2,
                        ln1_g, ln1_b, ln2_g, ln2_b)
    if _CACHED_NC is None:
        _CACHED_NC = build_block()
    nc = _CACHED_NC
    in_maps = [dict(wmap, x=np.ascontiguousarray(x[c])) for c in range(B)]
    res = run_bass_kernel_spmd(nc, in_maps, core_ids=list(range(B)),
                               trace=trace)
    LAST_RESULT = res
    out = np.stack([res.results[c]["out"] for c in range(B)])
    return out.astype(np.float32)
